# revision 14
# baseline (speedup 1.0000x reference)
"""GraphSAGE/GraphConv (DGL norm='both') Bass kernel for 8 Trainium2 cores.

Math (reference):
  x[n,f]   : node features, n in [0,160000), f in [0,64)   (from inputs[8,64,20000])
  agg[d]   = norm_dst[d] * sum_{e: dst[e]=d} norm_src[src[e]] * x[src[e]]
  out      = leaky_relu(agg @ W + b, 0.01), returned as [8,64,20000] feature-major.

Device strategy (per core, vertex-cut on dst):
  - core c owns dst nodes [c*20000,(c+1)*20000) == output slice c of dim 0.
  - norm_src is folded into the feature table on host (x_pre = x * norm_src,
    stored bf16 padded to 128 cols = 256B rows); norm_dst is applied after the
    W matmul (row scaling commutes with right-multiplication).
  - edges bucketed by (src window of 32768 rows, dst tile of 128 nodes), each
    bucket padded to 128-edge chunks; pad slots carry dl=255 so their one-hot
    row is all zeros.  One dma_gather per (tile-group, src window) pulls all
    the group's rows for that window (int16 window-local indices, 256B bf16
    rows), round-robined over 4 SWDGE queues.
  - DVE builds the one-hot S01[e,d]=(dst_local[e]==d); TensorE accumulates
    aggT[f,d] += G[e,:64]^T @ S01 in PSUM per tile across the 5 windows.
  - aggT evac (ACT, bf16), Z^T = W^T @ aggT (TensorE), ZT *= norm_dst (DVE,
    PSUM read), +bias (ACT Identity), leaky-relu batched in-place (DVE),
    two big DMAs for the [64,20000] slice.

All floating-point feature math runs on device; host does index/layout work
only (degrees -> norms, sort, padding, transpose/pad of the feature table).
"""

import os

import numpy as np
import ml_dtypes

from concourse import bass, mybir
import concourse.bacc as bacc
from concourse.tile import TileContext
from concourse.bass_utils import run_bass_kernel_spmd

BF16 = ml_dtypes.bfloat16
F32 = np.float32

LAST_RESULTS = None  # test harness introspection (exec time / trace)

CHUNK = 128     # edges per matmul chunk (PE contraction dim)
TW = 128        # dst-tile width (one-hot columns / PSUM free dim)
GRP = 4         # dst tiles per gather group
NQ = 1          # SWDGE queues for gather round-robin
WCLS = 32768    # src index window (int16 range for dma_gather)
ROWP = 128      # padded row width of the bf16 feature table (256B rows)
LEAK_SPAN = 8   # dst tiles per batched leaky-relu pass
CAPK = 4        # max chunks per dma_gather instruction
DL_PAD = 255.0  # sentinel dst-local for padded slots (no one-hot match)


def _build_layout(src, dst, n_nodes, n_cores, npc, wcls):
    """Slot layout: chunk m, partition p; arrays [128, M] per core, one shared
    shape. Class-major: for each src-window r, per-(tile,r) segments. Within
    each segment edges are sorted by src (better DRAM locality).

    Returns per-core idx16 (wrapped for dma_gather), dl slot array, the
    shared per-(class,tile) chunk counts and chunk-start table.
    """
    nt = -(-npc // TW)
    ncls = -(-n_nodes // wcls)

    owner = (dst // npc).astype(np.int64)
    rem = dst - owner * npc
    tile = rem // TW
    dl = rem - tile * TW
    r = (src // wcls).astype(np.int64)
    src_local = (src - r * wcls).astype(np.int16)

    # segment key: (core, class, tile); edges sorted by key then src
    key = (owner * ncls + r) * nt + tile
    order = np.lexsort((src, key))
    nseg = n_cores * ncls * nt
    counts = np.bincount(key, minlength=nseg).reshape(n_cores, ncls, nt)

    # shared chunk counts: max over cores, tile-0-class gets >=1 chunk
    chunks_rt = -(-counts.max(axis=0) // CHUNK)  # [ncls, nt]
    chunks_rt[0] = np.maximum(chunks_rt[0], 1)

    # chunk start per (r, t), class-major
    flat = chunks_rt.reshape(-1)
    seg_chunk_start = np.zeros(ncls * nt + 1, np.int64)
    np.cumsum(flat, out=seg_chunk_start[1:])
    M = int(seg_chunk_start[-1])

    # slot fill (vectorized over all edges)
    seg_start = np.zeros(nseg + 1, np.int64)
    np.cumsum(counts.reshape(-1), out=seg_start[1:])
    key_s = key[order]
    pos = np.arange(len(order), dtype=np.int64) - seg_start[key_s]
    rt_key = key_s % (ncls * nt)            # (r, t) within core
    c_s = key_s // (ncls * nt)
    m_s = seg_chunk_start[rt_key] + pos // CHUNK
    p_s = pos - (pos // CHUNK) * CHUNK

    idx_slot = np.zeros((n_cores, 128, M), np.int16)
    dl_all = np.full((n_cores, 128, M), DL_PAD, BF16)
    idx_slot[c_s, p_s, m_s] = src_local[order]
    dl_all[c_s, p_s, m_s] = dl[order].astype(BF16)

    # dma_gather index stream: flat order i = chunk-major (k*128+p), wrapped
    # into 16 partitions (idx16[j, s] = flat[s*16+j]) and replicated x8.
    # Column offset for chunk m is m*8.
    idx16 = np.zeros((n_cores, 128, M * (CHUNK // 16)), np.int16)
    for c in range(n_cores):
        flat_i = idx_slot[c].T.reshape(-1)            # [M*128], chunk-major
        wrapped = flat_i.reshape(-1, 16).T            # [16, M*8]
        idx16[c] = np.tile(wrapped, (8, 1))
    return idx16, dl_all, chunks_rt, seg_chunk_start, M, nt, ncls


def _build_nc(n_nodes, feat, outd, M, nt, npc, chunks_rt, seg_chunk_start,
              n_cores, wcls):
    f32 = mybir.dt.float32
    bf16 = mybir.dt.bfloat16
    i16 = mybir.dt.int16

    nc = bacc.Bacc(
        "TRN2",
        target_bir_lowering=False,
        debug=False,
        enable_asserts=False,
        num_devices=n_cores,
        num_swdge_queues=NQ,
    )

    ncls = chunks_rt.shape[0]
    scols = M * (CHUNK // 16)
    x_t = nc.dram_tensor("x_tab", [n_nodes, ROWP], bf16, kind="ExternalInput")
    idx_t = nc.dram_tensor("idx16", [128, scols], i16, kind="ExternalInput")
    dl_t = nc.dram_tensor("dl", [128, M], bf16, kind="ExternalInput")
    iota_t = nc.dram_tensor("iota", [128, TW], bf16, kind="ExternalInput")
    W_t = nc.dram_tensor("Wt", [feat, outd], bf16, kind="ExternalInput")
    b_t = nc.dram_tensor("bias", [outd, 1], f32, kind="ExternalInput")
    nd_t = nc.dram_tensor("ndst", [outd, nt * TW], bf16, kind="ExternalInput")
    out_t = nc.dram_tensor("out", [outd, npc], f32, kind="ExternalOutput")

    # tile groups
    groups = [list(range(g, min(g + GRP, nt))) for g in range(0, nt, GRP)]

    def rng(r, t0, t1):
        return int(seg_chunk_start[r * nt + t0]), int(seg_chunk_start[r * nt + t1 - 1] + chunks_rt[r, t1 - 1])

    kmax = max(rng(r, g[0], g[-1] + 1)[1] - rng(r, g[0], g[-1] + 1)[0]
               for r in range(ncls) for g in groups)

    gather_i = 0
    with TileContext(nc) as tc:
        with (
            tc.tile_pool(name="const", bufs=1) as constp,
            tc.tile_pool(name="gidx", bufs=3) as gidxp,
            tc.tile_pool(name="gbuf", bufs=3) as gpool,
            tc.tile_pool(name="onehot", bufs=3) as spool,
            tc.tile_pool(name="evac", bufs=4) as evacp,
            tc.tile_pool(name="zsb", bufs=4) as zsp,
            tc.tile_pool(name="leak", bufs=2) as lkp,
            tc.tile_pool(name="stage", bufs=2) as stagep,
            tc.tile_pool(name="psA", bufs=6, space="PSUM") as psA,
            tc.tile_pool(name="psZ", bufs=2, space="PSUM") as psZ,
        ):
            dl_sb = constp.tile([128, M], bf16)
            nc.sync.dma_start(dl_sb[:], dl_t[:])
            iota_sb = constp.tile([128, TW], bf16)
            nc.sync.dma_start(iota_sb[:], iota_t[:])
            W_sb = constp.tile([feat, outd], bf16)
            nc.sync.dma_start(W_sb[:], W_t[:])
            b_sb = constp.tile([outd, 1], f32)
            nc.sync.dma_start(b_sb[:], b_t[:])
            nd_sb = constp.tile([outd, nt * TW], bf16)
            nc.sync.dma_start(nd_sb[:], nd_t[:])

            half = (len(groups) + 1) // 2
            ghalves = [groups[:half], groups[half:]]
            for gh in ghalves:
                t_lo = gh[0][0]
                n_cols_h = (gh[-1][-1] + 1 - t_lo) * TW
                stage = stagep.tile([outd, ((nt + 1) // 2 + GRP) * TW], f32, tag="st")
                for grp in gh:
                    aggs = {}
                    for t in grp:
                        aggs[t] = psA.tile([feat, TW], f32,
                                           name=f"agg{t}", tag="agg")
                    first = {t: True for t in grp}
                    last_r = {t: max(r for r in range(ncls)
                                     if chunks_rt[r, t] > 0) for t in grp}
                    for r in range(ncls):
                      ga, gb = rng(r, grp[0], grp[-1] + 1)
                      for a in range(ga, gb, CAPK):
                        b = min(a + CAPK, gb)
                        K = b - a
                        if K == 0:
                            continue
                        row0 = r * wcls
                        row1 = min(n_nodes, (r + 1) * wcls)
                        it = gidxp.tile([128, K * (CHUNK // 16)], i16, tag="gi")
                        nc.sync.dma_start(
                            it[:],
                            idx_t[:, a * (CHUNK // 16):b * (CHUNK // 16)],
                        )
                        gt = gpool.tile([128, K * ROWP], bf16, tag="g")
                        nc.gpsimd.dma_gather(
                            out_ap=gt[:].rearrange(
                                "p (k f) -> p k f", f=ROWP),
                            in_ap=x_t[row0:row1, :],
                            idxs_ap=it[:],
                            num_idxs=K * CHUNK,
                            num_idxs_reg=K * CHUNK,
                            elem_size=ROWP,
                            queue_num=gather_i % NQ,
                        )
                        gather_i += 1
                        st = spool.tile([128, K * TW], bf16, tag="s")
                        nc.vector.tensor_tensor(
                            out=st[:].rearrange(
                                "p (k d) -> p k d", d=TW),
                            in0=iota_sb[:]
                            .rearrange("p (o d) -> p o d", o=1)
                            .to_broadcast([128, K, TW]),
                            in1=dl_sb[:, a:b]
                            .rearrange("p (k o) -> p k o", o=1)
                            .to_broadcast([128, K, TW]),
                            op=mybir.AluOpType.is_equal,
                        )
                        for t in grp:
                            s0 = int(seg_chunk_start[r * nt + t])
                            ct = int(chunks_rt[r, t])
                            tl = t - grp[0]
                            for m in range(max(s0, a), min(s0 + ct, b)):
                                col = m - a
                                nc.tensor.matmul(
                                    out=aggs[t][:],
                                    lhsT=gt[:, col * ROWP:col * ROWP + feat],
                                    rhs=st[:, col * TW:(col + 1) * TW],
                                    start=first[t],
                                    stop=(r == last_r[t] and m == s0 + ct - 1),
                                )
                                first[t] = False
                    gw = len(grp) * TW
                    agg_sb = evacp.tile([feat, GRP * TW], bf16, tag="ev")
                    for tl, t in enumerate(grp):
                        nc.scalar.activation(
                            agg_sb[:, tl * TW:(tl + 1) * TW], aggs[t][:],
                            mybir.ActivationFunctionType.Copy,
                        )
                    ZB = psZ.tile([outd, GRP * TW], f32, tag="z")
                    for tl, t in enumerate(grp):
                        nc.tensor.matmul(
                            out=ZB[:, tl * TW:(tl + 1) * TW],
                            lhsT=W_sb[:],
                            rhs=agg_sb[:, tl * TW:(tl + 1) * TW],
                            start=True, stop=True,
                        )
                    zs = zsp.tile([outd, GRP * TW], f32, tag="zs")
                    nc.vector.tensor_tensor(
                        out=zs[:, :gw],
                        in0=ZB[:, :gw],
                        in1=nd_sb[:, grp[0] * TW:grp[0] * TW + gw],
                        op=mybir.AluOpType.mult,
                    )
                    lt0 = grp[0] - t_lo
                    nc.scalar.activation(
                        stage[:, lt0 * TW:lt0 * TW + gw],
                        zs[:, :gw],
                        mybir.ActivationFunctionType.Identity,
                        bias=b_sb[:],
                    )
                    span = stage[:, lt0 * TW:lt0 * TW + gw]
                    zl = lkp.tile([outd, GRP * TW], f32, tag="zl")
                    nc.vector.tensor_scalar(
                        out=zl[:, :gw],
                        in0=span,
                        scalar1=0.01,
                        scalar2=None,
                        op0=mybir.AluOpType.mult,
                    )
                    nc.vector.tensor_tensor(
                        out=span,
                        in0=span,
                        in1=zl[:, :gw],
                        op=mybir.AluOpType.max,
                    )
                out_cols = min(npc, (gh[-1][-1] + 1) * TW) - t_lo * TW
                nc.sync.dma_start(
                    out_t[:, t_lo * TW:t_lo * TW + out_cols],
                    stage[:, :out_cols],
                )

    nc.compile()
    return nc


def _prep(inputs, W, b, src, dst, n_cores, wcls=WCLS):
    sli, feat, node = inputs.shape
    n_nodes = sli * node
    outd = W.shape[1]
    npc = n_nodes // n_cores

    src = np.asarray(src).astype(np.int64)
    dst = np.asarray(dst).astype(np.int64)
    deg_out = np.bincount(src, minlength=n_nodes)
    deg_in = np.bincount(dst, minlength=n_nodes)
    norm_src = np.maximum(deg_out, 1).astype(F32) ** -0.5
    norm_dst = np.maximum(deg_in, 1).astype(F32) ** -0.5

    x_flat = np.asarray(inputs, dtype=F32).transpose(0, 2, 1).reshape(
        n_nodes, feat)
    x_pre = np.zeros((n_nodes, ROWP), BF16)
    x_pre[:, :feat] = (x_flat * norm_src[:, None]).astype(BF16)

    idx16, dl_all, chunks_rt, seg_chunk_start, M, nt, ncls = _build_layout(
        src, dst, n_nodes, n_cores, npc, wcls
    )

    iota = np.broadcast_to(np.arange(TW, dtype=F32), (128, TW)).astype(BF16)
    Wt = np.asarray(W, dtype=F32).astype(BF16)
    bias = np.asarray(b, dtype=F32).reshape(outd, 1)
    nd_bf = norm_dst.astype(BF16)

    in_maps = []
    npc_pad = nt * TW
    for c in range(n_cores):
        nd_pad = np.zeros(npc_pad, BF16)
        nd_pad[:npc] = nd_bf[c * npc:(c + 1) * npc]
        ndst_rep = np.ascontiguousarray(
            np.broadcast_to(nd_pad, (outd, npc_pad))
        )
        in_maps.append(
            {
                "x_tab": x_pre,
                "idx16": np.ascontiguousarray(idx16[c]),
                "dl": np.ascontiguousarray(dl_all[c]),
                "iota": np.ascontiguousarray(iota),
                "Wt": Wt,
                "bias": bias,
                "ndst": ndst_rep,
            }
        )
    meta = dict(
        n_nodes=n_nodes, feat=feat, outd=outd, M=M, nt=nt, npc=npc,
        chunks_rt=chunks_rt, seg_chunk_start=seg_chunk_start,
        sli=sli, node=node, wcls=wcls,
    )
    return in_maps, meta


def kernel(inputs, W, b, src, dst):
    global LAST_RESULTS
    n_cores = 8
    inputs = np.asarray(inputs, dtype=F32)
    in_maps, meta = _prep(inputs, W, b, src, dst, n_cores)

    nc = _build_nc(
        meta["n_nodes"], meta["feat"], meta["outd"], meta["M"], meta["nt"],
        meta["npc"], meta["chunks_rt"], meta["seg_chunk_start"], n_cores,
        meta["wcls"],
    )

    res = run_bass_kernel_spmd(
        nc,
        in_maps,
        core_ids=list(range(n_cores)),
        trace=bool(int(os.environ.get("KERNEL_TRACE", "0"))),
    )
    LAST_RESULTS = res

    out = np.stack([r["out"] for r in res.results], axis=0)  # [8, 64, 20000]
    return out.astype(F32)


# revision 16
# speedup vs baseline: 1.0831x; 1.0831x over previous
"""GraphSAGE/GraphConv (DGL norm='both') Bass kernel for 8 Trainium2 cores.

Math (reference):
  x[n,f]   : node features, n in [0,160000), f in [0,64)   (from inputs[8,64,20000])
  agg[d]   = norm_dst[d] * sum_{e: dst[e]=d} norm_src[src[e]] * x[src[e]]
  out      = leaky_relu(agg @ W + b, 0.01), returned as [8,64,20000] feature-major.

Device strategy (per core, vertex-cut on dst):
  - core c owns dst nodes [c*20000,(c+1)*20000) == output slice c of dim 0.
  - norm_src is folded into the feature table on host (x_pre = x * norm_src,
    stored bf16 padded to 128 cols = 256B rows); norm_dst is applied after the
    W matmul (row scaling commutes with right-multiplication).
  - edges bucketed by (src window of 32768 rows, dst tile of 128 nodes), each
    bucket padded to 128-edge chunks; pad slots carry dl=255 so their one-hot
    row is all zeros.  One dma_gather per (tile-group, src window) pulls all
    the group's rows for that window (int16 window-local indices, 256B bf16
    rows), round-robined over 4 SWDGE queues.
  - DVE builds the one-hot S01[e,d]=(dst_local[e]==d); TensorE accumulates
    aggT[f,d] += G[e,:64]^T @ S01 in PSUM per tile across the 5 windows.
  - aggT evac (ACT, bf16), Z^T = W^T @ aggT (TensorE), ZT *= norm_dst (DVE,
    PSUM read), +bias (ACT Identity), leaky-relu batched in-place (DVE),
    two big DMAs for the [64,20000] slice.

All floating-point feature math runs on device; host does index/layout work
only (degrees -> norms, sort, padding, transpose/pad of the feature table).
"""

import os

import numpy as np
import ml_dtypes

from concourse import bass, mybir
import concourse.bacc as bacc
from concourse.tile import TileContext
from concourse.bass_utils import run_bass_kernel_spmd

BF16 = ml_dtypes.bfloat16
F32 = np.float32

LAST_RESULTS = None  # test harness introspection (exec time / trace)

CHUNK = 128     # edges per matmul chunk (PE contraction dim)
TW = 128        # dst-tile width (one-hot columns / PSUM free dim)
GRP = 4         # dst tiles per gather group
NQ = 1          # SWDGE queues for gather round-robin
WCLS = 32768    # src index window (int16 range for dma_gather)
ROWP = 128      # padded row width of the bf16 feature table (256B rows)
LEAK_SPAN = 8   # dst tiles per batched leaky-relu pass
CAPK = 8        # max chunks per dma_gather instruction
DL_PAD = 255.0  # sentinel dst-local for padded slots (no one-hot match)


def _build_layout(src, dst, n_nodes, n_cores, npc, wcls):
    """Slot layout: chunk m, partition p; arrays [128, M] per core, one shared
    shape. Class-major: for each src-window r, per-(tile,r) segments. Within
    each segment edges are sorted by src (better DRAM locality).

    Returns per-core idx16 (wrapped for dma_gather), dl slot array, the
    shared per-(class,tile) chunk counts and chunk-start table.
    """
    nt = -(-npc // TW)
    ncls = -(-n_nodes // wcls)

    owner = (dst // npc).astype(np.int64)
    rem = dst - owner * npc
    tile = rem // TW
    dl = rem - tile * TW
    r = (src // wcls).astype(np.int64)
    src_local = (src - r * wcls).astype(np.int16)

    # segment key: (core, class, tile); edges sorted by key then src
    key = (owner * ncls + r) * nt + tile
    order = np.lexsort((src, key))
    nseg = n_cores * ncls * nt
    counts = np.bincount(key, minlength=nseg).reshape(n_cores, ncls, nt)

    # shared chunk counts: max over cores, tile-0-class gets >=1 chunk
    chunks_rt = -(-counts.max(axis=0) // CHUNK)  # [ncls, nt]
    chunks_rt[0] = np.maximum(chunks_rt[0], 1)

    # chunk start per (r, t), class-major
    flat = chunks_rt.reshape(-1)
    seg_chunk_start = np.zeros(ncls * nt + 1, np.int64)
    np.cumsum(flat, out=seg_chunk_start[1:])
    M = int(seg_chunk_start[-1])

    # slot fill (vectorized over all edges)
    seg_start = np.zeros(nseg + 1, np.int64)
    np.cumsum(counts.reshape(-1), out=seg_start[1:])
    key_s = key[order]
    pos = np.arange(len(order), dtype=np.int64) - seg_start[key_s]
    rt_key = key_s % (ncls * nt)            # (r, t) within core
    c_s = key_s // (ncls * nt)
    m_s = seg_chunk_start[rt_key] + pos // CHUNK
    p_s = pos - (pos // CHUNK) * CHUNK

    idx_slot = np.zeros((n_cores, 128, M), np.int16)
    dl_all = np.full((n_cores, 128, M), DL_PAD, BF16)
    idx_slot[c_s, p_s, m_s] = src_local[order]
    dl_all[c_s, p_s, m_s] = dl[order].astype(BF16)
    fills = np.zeros((n_cores, M), np.int64)
    np.add.at(fills, (c_s, m_s), 1)

    # dma_gather index stream: flat order i = chunk-major (k*128+p), wrapped
    # into 16 partitions (idx16[j, s] = flat[s*16+j]) and replicated x8.
    # Column offset for chunk m is m*8.
    idx16 = np.zeros((n_cores, 128, M * (CHUNK // 16)), np.int16)
    for c in range(n_cores):
        flat_i = idx_slot[c].T.reshape(-1)            # [M*128], chunk-major
        wrapped = flat_i.reshape(-1, 16).T            # [16, M*8]
        idx16[c] = np.tile(wrapped, (8, 1))
    return idx16, dl_all, chunks_rt, seg_chunk_start, fills, M, nt, ncls


def _build_nc(n_nodes, feat, outd, M, nt, npc, chunks_rt, seg_chunk_start,
              fills, n_cores, wcls):
    f32 = mybir.dt.float32
    bf16 = mybir.dt.bfloat16
    i16 = mybir.dt.int16

    nc = bacc.Bacc(
        "TRN2",
        target_bir_lowering=False,
        debug=False,
        enable_asserts=False,
        num_devices=n_cores,
        num_swdge_queues=NQ,
    )

    ncls = chunks_rt.shape[0]
    scols = M * (CHUNK // 16)
    x_t = nc.dram_tensor("x_tab", [n_nodes, ROWP], bf16, kind="ExternalInput")
    idx_t = nc.dram_tensor("idx16", [128, scols], i16, kind="ExternalInput")
    dl_t = nc.dram_tensor("dl", [128, M], bf16, kind="ExternalInput")
    iota_t = nc.dram_tensor("iota", [128, TW], bf16, kind="ExternalInput")
    W_t = nc.dram_tensor("Wt", [feat, outd], bf16, kind="ExternalInput")
    b_t = nc.dram_tensor("bias", [outd, 1], f32, kind="ExternalInput")
    nd_t = nc.dram_tensor("ndst", [outd, nt * TW], bf16, kind="ExternalInput")
    out_t = nc.dram_tensor("out", [outd, npc], f32, kind="ExternalOutput")

    # tile groups
    groups = [list(range(g, min(g + GRP, nt))) for g in range(0, nt, GRP)]

    def rng(r, t0, t1):
        return int(seg_chunk_start[r * nt + t0]), int(seg_chunk_start[r * nt + t1 - 1] + chunks_rt[r, t1 - 1])

    kmax = max(rng(r, g[0], g[-1] + 1)[1] - rng(r, g[0], g[-1] + 1)[0]
               for r in range(ncls) for g in groups)

    gather_i = 0
    with TileContext(nc) as tc:
        with (
            tc.tile_pool(name="const", bufs=1) as constp,
            tc.tile_pool(name="gidx", bufs=3) as gidxp,
            tc.tile_pool(name="gbuf", bufs=3) as gpool,
            tc.tile_pool(name="onehot", bufs=3) as spool,
            tc.tile_pool(name="evac", bufs=4) as evacp,
            tc.tile_pool(name="zsb", bufs=4) as zsp,
            tc.tile_pool(name="leak", bufs=2) as lkp,
            tc.tile_pool(name="stage", bufs=2) as stagep,
            tc.tile_pool(name="psA", bufs=6, space="PSUM") as psA,
            tc.tile_pool(name="psZ", bufs=2, space="PSUM") as psZ,
        ):
            dl_sb = constp.tile([128, M], bf16)
            nc.sync.dma_start(dl_sb[:], dl_t[:])
            iota_sb = constp.tile([128, TW], bf16)
            nc.sync.dma_start(iota_sb[:], iota_t[:])
            W_sb = constp.tile([feat, outd], bf16)
            nc.sync.dma_start(W_sb[:], W_t[:])
            b_sb = constp.tile([outd, 1], f32)
            nc.sync.dma_start(b_sb[:], b_t[:])
            nd_sb = constp.tile([outd, nt * TW], bf16)
            nc.sync.dma_start(nd_sb[:], nd_t[:])

            for wi in range(3):
                gwarm = gpool.tile([128, CAPK * ROWP], bf16,
                                   name=f"gwarm{wi}", tag="g")
                nc.vector.memset(gwarm[:], 0)

            half = (len(groups) + 1) // 2
            ghalves = [groups[:half], groups[half:]]
            for gh in ghalves:
                t_lo = gh[0][0]
                n_cols_h = (gh[-1][-1] + 1 - t_lo) * TW
                stage = stagep.tile([outd, ((nt + 1) // 2 + GRP) * TW], f32, tag="st")
                for grp in gh:
                    aggs = {}
                    for t in grp:
                        aggs[t] = psA.tile([feat, TW], f32,
                                           name=f"agg{t}", tag="agg")
                    first = {t: True for t in grp}
                    last_r = {t: max(r for r in range(ncls)
                                     if chunks_rt[r, t] > 0) for t in grp}
                    for r in range(ncls):
                      ga, gb = rng(r, grp[0], grp[-1] + 1)
                      for a in range(ga, gb, CAPK):
                        b = min(a + CAPK, gb)
                        K = b - a
                        if K == 0:
                            continue
                        row0 = r * wcls
                        row1 = min(n_nodes, (r + 1) * wcls)
                        it = gidxp.tile([128, K * (CHUNK // 16)], i16, tag="gi")
                        nc.sync.dma_start(
                            it[:],
                            idx_t[:, a * (CHUNK // 16):b * (CHUNK // 16)],
                        )
                        f_last = int(fills[:, b - 1].max())
                        nid = (K - 1) * CHUNK + max(f_last, 1)
                        nid = min(-(-nid // 16) * 16, K * CHUNK)
                        gt = gpool.tile([128, K * ROWP], bf16, tag="g")
                        nc.gpsimd.dma_gather(
                            out_ap=gt[:].rearrange(
                                "p (k f) -> p k f", f=ROWP),
                            in_ap=x_t[row0:row1, :],
                            idxs_ap=it[:],
                            num_idxs=nid,
                            num_idxs_reg=nid,
                            elem_size=ROWP,
                            queue_num=gather_i % NQ,
                        )
                        gather_i += 1
                        st = spool.tile([128, K * TW], bf16, tag="s")
                        nc.vector.tensor_tensor(
                            out=st[:].rearrange(
                                "p (k d) -> p k d", d=TW),
                            in0=iota_sb[:]
                            .rearrange("p (o d) -> p o d", o=1)
                            .to_broadcast([128, K, TW]),
                            in1=dl_sb[:, a:b]
                            .rearrange("p (k o) -> p k o", o=1)
                            .to_broadcast([128, K, TW]),
                            op=mybir.AluOpType.is_equal,
                        )
                        for t in grp:
                            s0 = int(seg_chunk_start[r * nt + t])
                            ct = int(chunks_rt[r, t])
                            tl = t - grp[0]
                            for m in range(max(s0, a), min(s0 + ct, b)):
                                col = m - a
                                nc.tensor.matmul(
                                    out=aggs[t][:],
                                    lhsT=gt[:, col * ROWP:col * ROWP + feat],
                                    rhs=st[:, col * TW:(col + 1) * TW],
                                    start=first[t],
                                    stop=(r == last_r[t] and m == s0 + ct - 1),
                                )
                                first[t] = False
                    gw = len(grp) * TW
                    agg_sb = evacp.tile([feat, GRP * TW], bf16, tag="ev")
                    for tl, t in enumerate(grp):
                        nc.scalar.activation(
                            agg_sb[:, tl * TW:(tl + 1) * TW], aggs[t][:],
                            mybir.ActivationFunctionType.Copy,
                        )
                    ZB = psZ.tile([outd, GRP * TW], f32, tag="z")
                    for tl, t in enumerate(grp):
                        nc.tensor.matmul(
                            out=ZB[:, tl * TW:(tl + 1) * TW],
                            lhsT=W_sb[:],
                            rhs=agg_sb[:, tl * TW:(tl + 1) * TW],
                            start=True, stop=True,
                        )
                    zs = zsp.tile([outd, GRP * TW], f32, tag="zs")
                    nc.vector.tensor_tensor(
                        out=zs[:, :gw],
                        in0=ZB[:, :gw],
                        in1=nd_sb[:, grp[0] * TW:grp[0] * TW + gw],
                        op=mybir.AluOpType.mult,
                    )
                    lt0 = grp[0] - t_lo
                    nc.scalar.activation(
                        stage[:, lt0 * TW:lt0 * TW + gw],
                        zs[:, :gw],
                        mybir.ActivationFunctionType.Identity,
                        bias=b_sb[:],
                    )
                    span = stage[:, lt0 * TW:lt0 * TW + gw]
                    zl = lkp.tile([outd, GRP * TW], f32, tag="zl")
                    nc.vector.tensor_scalar(
                        out=zl[:, :gw],
                        in0=span,
                        scalar1=0.01,
                        scalar2=None,
                        op0=mybir.AluOpType.mult,
                    )
                    nc.vector.tensor_tensor(
                        out=span,
                        in0=span,
                        in1=zl[:, :gw],
                        op=mybir.AluOpType.max,
                    )
                out_cols = min(npc, (gh[-1][-1] + 1) * TW) - t_lo * TW
                nc.sync.dma_start(
                    out_t[:, t_lo * TW:t_lo * TW + out_cols],
                    stage[:, :out_cols],
                )

    nc.compile()
    return nc


def _prep(inputs, W, b, src, dst, n_cores, wcls=WCLS):
    sli, feat, node = inputs.shape
    n_nodes = sli * node
    outd = W.shape[1]
    npc = n_nodes // n_cores

    src = np.asarray(src).astype(np.int64)
    dst = np.asarray(dst).astype(np.int64)
    deg_out = np.bincount(src, minlength=n_nodes)
    deg_in = np.bincount(dst, minlength=n_nodes)
    norm_src = np.maximum(deg_out, 1).astype(F32) ** -0.5
    norm_dst = np.maximum(deg_in, 1).astype(F32) ** -0.5

    x_flat = np.asarray(inputs, dtype=F32).transpose(0, 2, 1).reshape(
        n_nodes, feat)
    x_pre = np.zeros((n_nodes, ROWP), BF16)
    x_pre[:, :feat] = (x_flat * norm_src[:, None]).astype(BF16)

    idx16, dl_all, chunks_rt, seg_chunk_start, fills, M, nt, ncls = _build_layout(
        src, dst, n_nodes, n_cores, npc, wcls
    )

    iota = np.broadcast_to(np.arange(TW, dtype=F32), (128, TW)).astype(BF16)
    Wt = np.asarray(W, dtype=F32).astype(BF16)
    bias = np.asarray(b, dtype=F32).reshape(outd, 1)
    nd_bf = norm_dst.astype(BF16)

    in_maps = []
    npc_pad = nt * TW
    for c in range(n_cores):
        nd_pad = np.zeros(npc_pad, BF16)
        nd_pad[:npc] = nd_bf[c * npc:(c + 1) * npc]
        ndst_rep = np.ascontiguousarray(
            np.broadcast_to(nd_pad, (outd, npc_pad))
        )
        in_maps.append(
            {
                "x_tab": x_pre,
                "idx16": np.ascontiguousarray(idx16[c]),
                "dl": np.ascontiguousarray(dl_all[c]),
                "iota": np.ascontiguousarray(iota),
                "Wt": Wt,
                "bias": bias,
                "ndst": ndst_rep,
            }
        )
    meta = dict(
        n_nodes=n_nodes, feat=feat, outd=outd, M=M, nt=nt, npc=npc,
        chunks_rt=chunks_rt, seg_chunk_start=seg_chunk_start, fills=fills,
        sli=sli, node=node, wcls=wcls,
    )
    return in_maps, meta


def kernel(inputs, W, b, src, dst):
    global LAST_RESULTS
    n_cores = 8
    inputs = np.asarray(inputs, dtype=F32)
    in_maps, meta = _prep(inputs, W, b, src, dst, n_cores)

    nc = _build_nc(
        meta["n_nodes"], meta["feat"], meta["outd"], meta["M"], meta["nt"],
        meta["npc"], meta["chunks_rt"], meta["seg_chunk_start"],
        meta["fills"], n_cores, meta["wcls"],
    )

    res = run_bass_kernel_spmd(
        nc,
        in_maps,
        core_ids=list(range(n_cores)),
        trace=bool(int(os.environ.get("KERNEL_TRACE", "0"))),
    )
    LAST_RESULTS = res

    out = np.stack([r["out"] for r in res.results], axis=0)  # [8, 64, 20000]
    return out.astype(F32)
